# revision 23
# baseline (speedup 1.0000x reference)
"""Fused bidirectional (ESIM) attention kernel for Trainium2 (Bass/Tile).

Problem: B=16, Lp=Lh=2048, D=256 fp32.
  sim = P @ H^T / sqrt(D)
  attended_premises   = masked_softmax(sim,   hm) @ H * pm
  attended_hypotheses = masked_softmax(sim^T, pm) @ P * hm

Key identities / tricks:
  - softmax(scores*mask)*mask / (sum + EPS) reduces to
    out_j = e_j*m_j / sum_k e_k*m_k with e = exp(scores).
  - Masks are dense 0/1 with p=0.5: the host COMPACTS each sequence to its
    valid rows (padded to Lv = max valid count rounded up to 128). All
    device matmuls run on ~Lv=1152 instead of 2048 rows per side (~3.2x
    less PE work; PE is the bottleneck). Outputs scatter back on the host.
  - bf16 operands (fp32 PSUM accumulation): same PE rate as float32r but
    half the DMA traffic; rel-err ~5e-3, inside the 2e-2 gate.
  - Scores are computed ONCE (orientation [h, p]). The second direction's
    E^T comes from dma_start_transpose (DMA xbar, SBUF->SBUF, block-major
    fold [128, G*128] -> [128, G, 128] verified on HW), saving 25% of PE
    work vs recomputing the scores transposed. E chunks are grouped 3 per
    tile so one transpose instruction covers 3 chunks (the per-instruction
    queue cost is what hurts).
  - Both directions' softmax denominators ride the weighted-sum matmuls
    via two valid-indicator columns appended to the compacted K-side rows
    (N=258); no separate reduction anywhere.
  - DMA queue discipline: the SP queue pays ~5ns per descriptor, so loads
    are 2 fat DMAs per batch and outputs are staged in SBUF and written as
    3-chunk contiguous DMAs (128 x 3KB descriptors instead of 1KB rows).
  - A PE warmup burst of dummy matmuls runs during the ~8us DMA/queue
    startup so real matmuls start at the HAM-warm 2.4 GHz clock.

Sharding: data-parallel over batch, 2 batches per core on 8 cores.

Per batch (Lv padded valid length, NC = Lv/128 chunks, GC = NC/3 groups):
  A1: V1[h,p] per 128-row h-chunk; exp -> E1 chunk (bf16, grouped 3/tile);
      per group one dma-transpose into the E2 buffer [128, 27, 128].
  B1: per p-tile: acc[p, :258] = sum_h E1ᵀ @ [Hc | 1 1]; out = acc[:, :256]
      * 1/acc[:, 256] into a staging tile, 3-chunk DMAs out.
  B2: same with E2 group slices and [Pc | 1 1].
Emission order A1(0) B1(0) A1(1) B2(0) B1(1) B2(1) keeps the PE FIFO from
head-of-line blocking on batch-0 transposes.
"""

import numpy as np
import ml_dtypes

import concourse.mybir as mybir
import concourse.tile as tile
from concourse import bacc
from concourse.bass_utils import run_bass_kernel_spmd

F32 = mybir.dt.float32
BF16 = mybir.dt.bfloat16
EXP = mybir.ActivationFunctionType.Exp
BF16NP = ml_dtypes.bfloat16

B, L, D = 16, 2048, 256
NCORES = 8
BPC = B // NCORES      # batches per core
DC = D // 128          # 2 contraction chunks of 128 for the score matmuls
NAUG = D + 2           # compacted K-side rows + two valid-indicator columns
GRP = 3                # E chunks per transpose group
SCALE = 1.0 / np.sqrt(np.float32(D)).astype(np.float32)


def PSW(Lv):
    """Score-PSUM tile width, rounded to a whole PSUM bank multiple."""
    return ((Lv * 4 + 2047) // 2048) * 512


def _scores(nc, ldA, Lv, Nq, psv, ep, e2p, tag):
    """Single score pass: V1[h,p] per h-chunk, exp into grouped E tiles,
    one dma-transpose per group into the E2 chunk-folded layout.

    Score matmuls only cover query columns [0, Nq) (the max valid premise
    count): E columns beyond hold exp(stale PSUM), which is bounded (PSUM
    score banks only ever contain zeros from warmup or old score values),
    and they only feed dropped output partitions / zeroed contractions.

    Returns (E1 group tiles, E2 group tiles)."""
    NC = Lv // 128
    GC = NC // GRP
    psw = PSW(Lv)
    la0, la1 = ldA
    xth = [la0[:, 0:Lv], la1[:, 0:Lv]]
    xtp = [la0[:, Lv : 2 * Lv], la1[:, Lv : 2 * Lv]]
    e1g, e2g = [], []
    for g in range(GC):
        e1g.append(ep.tile([128, GRP, Lv], BF16, tag=f"E{g}", name=f"E{tag}_{g}"))
        e2g.append(
            e2p.tile([128, GRP * NC, 128], BF16, tag=f"T{g}", name=f"T{tag}_{g}")
        )
    for kc in range(NC):
        ps = psv.tile([128, psw], F32, tag="v", name=f"v{tag}{kc}")
        for dc in range(DC):
            for off in range(0, Nq, 512):
                w = min(512, Nq - off)
                nc.tensor.matmul(
                    ps[:, off : off + w],
                    lhsT=xth[dc][:, kc * 128 : (kc + 1) * 128],
                    rhs=xtp[dc][:, off : off + w],
                    start=(dc == 0),
                    stop=(dc == DC - 1),
                )
        g, r = divmod(kc, GRP)
        nc.scalar.activation(e1g[g][:, r, :], ps[:, 0:Lv], EXP, scale=float(SCALE))
        if r == GRP - 1:
            nc.sync.dma_start_transpose(e2g[g][:, :, :], e1g[g][:, :, :])
    return e1g, e2g


def _wsum(nc, lhs_of, xa, out_dram, Lv, pac, den, outp, tag):
    """One direction's weighted sum + normalize + per-tile output DMAs.

    lhs_of(qt, kc) -> [128, 128] bf16 lhsT slice (contraction chunk kc for
    query tile qt). xa: [128, NC*NAUG] K-side rows with valid columns.
    Output DMAs issue from the ACT hwdge queue (its emission position there
    matches PE order, and the Sync queue is busy with the E transposes)."""
    NC = Lv // 128
    for qt in range(NC):
        acc = pac.tile([128, 512], F32, tag="acc", name=f"acc{tag}_{qt}")
        for kc in range(NC):
            nc.tensor.matmul(
                acc[:, 0:NAUG],
                lhsT=lhs_of(qt, kc),
                rhs=xa[:, kc * NAUG : (kc + 1) * NAUG],
                start=(kc == 0),
                stop=(kc == NC - 1),
            )
        r = den.tile([128, 1], F32, tag="rec", name=f"rec{tag}_{qt}")
        nc.vector.reciprocal(r[:], acc[:, D : D + 1])
        ot = outp.tile([128, D], F32, tag="ot", name=f"ot{tag}_{qt}")
        nc.vector.tensor_scalar_mul(ot[:], acc[:, 0:D], r[:])
        nc.scalar.dma_start(
            out=out_dram[:, qt * D : (qt + 1) * D], in_=ot[:]
        )


def build_program(Lv, Nq, bpc=BPC):
    NC = Lv // 128
    assert NC % GRP == 0
    nc = bacc.Bacc("TRN2", target_bir_lowering=False, debug=False, num_devices=NCORES)
    # ldA: the four d-major score operands [xth0|xtp0|xth1|xtp1].
    # ldB: the two augmented K-side row blocks [xAh|xAp].
    ldA_t = nc.dram_tensor("ldA", [bpc, 128, 4 * Lv], BF16, kind="ExternalInput").ap()
    ldB_t = nc.dram_tensor(
        "ldB", [bpc, 128, 2 * NC * NAUG], BF16, kind="ExternalInput"
    ).ap()
    out_p = nc.dram_tensor("out_prem", [bpc, 128, NC * D], F32, kind="ExternalOutput").ap()
    out_h = nc.dram_tensor("out_hyp", [bpc, 128, NC * D], F32, kind="ExternalOutput").ap()

    with tile.TileContext(nc) as tc:
        with (
            tc.tile_pool(name="tp", bufs=2) as tp,
            tc.tile_pool(name="ep", bufs=2) as ep,
            tc.tile_pool(name="e2p", bufs=2) as e2p,
            tc.tile_pool(name="outp", bufs=4) as outp,
            tc.tile_pool(name="psv", bufs=2, space="PSUM") as psv,
            tc.tile_pool(name="pac", bufs=2, space="PSUM") as pac,
            tc.tile_pool(name="den", bufs=4) as den,
        ):
            # PE warmup: engine queues + the first operand DMA take ~13us;
            # run dummy matmuls meanwhile so HAM un-throttles before real
            # work. The dummies also sweep every (buffer, bank) region of
            # the score-PSUM pool, zeroing whatever a previous NEFF left
            # there (keeps exp(stale PSUM) in the skipped pad columns
            # finite forever after).
            wuw = tp.tile([128, 128], BF16, tag="wuw", name="wuw")
            nc.vector.memset(wuw[:], 0)
            wur = tp.tile([128, 512], BF16, tag="wur", name="wur")
            nc.vector.memset(wur[:], 0)
            nreg = PSW(Lv) // 512
            for i in range(9):
                pswu = psv.tile([128, PSW(Lv)], F32, tag="v", name=f"wu{i}")
                off = ((i // 2) % nreg) * 512
                nc.tensor.matmul(
                    pswu[:, off : off + 512], lhsT=wuw[:], rhs=wur[:],
                    start=True, stop=True,
                )

            def loads(b):
                # Split the score operands in two so the dc=0 pair lands
                # ~1.5us earlier than one fat transfer would.
                la0 = tp.tile([128, 2 * Lv], BF16, tag="ldA0", name=f"ldA0_{b}")
                nc.sync.dma_start(out=la0[:], in_=ldA_t[b, :, 0 : 2 * Lv])
                la1 = tp.tile([128, 2 * Lv], BF16, tag="ldA1", name=f"ldA1_{b}")
                nc.sync.dma_start(out=la1[:], in_=ldA_t[b, :, 2 * Lv : 4 * Lv])
                lb = tp.tile([128, 2 * NC * NAUG], BF16, tag="ldB", name=f"ldB{b}")
                nc.sync.dma_start(out=lb[:], in_=ldB_t[b])
                return (la0, la1), lb

            def d1_lhs(e1g):
                def f(qt, kc):
                    return e1g[kc // GRP][:, kc % GRP, qt * 128 : (qt + 1) * 128]
                return f

            def d2_lhs(e2g):
                def f(qt, kc):
                    # e2 group tile j-index = (h-chunk within group)*NC + pc
                    return e2g[qt // GRP][:, (qt % GRP) * NC + kc, :]
                return f

            st = [None] * bpc
            eg = [None] * bpc
            st[0] = loads(0)
            eg[0] = _scores(nc, st[0][0], Lv, Nq, psv, ep, e2p, "0")
            _wsum(
                nc, d1_lhs(eg[0][0]), st[0][1][:, 0 : NC * NAUG], out_p[0],
                Lv, pac, den, outp, "p0",
            )
            for b in range(bpc):
                if b + 1 < bpc:
                    st[b + 1] = loads(b + 1)
                    eg[b + 1] = _scores(
                        nc, st[b + 1][0], Lv, Nq, psv, ep, e2p, f"{b+1}"
                    )
                _wsum(
                    nc, d2_lhs(eg[b][1]), st[b][1][:, NC * NAUG :], out_h[b],
                    Lv, pac, den, outp, f"h{b}",
                )
                if b + 1 < bpc:
                    _wsum(
                        nc, d1_lhs(eg[b + 1][0]), st[b + 1][1][:, 0 : NC * NAUG],
                        out_p[b + 1], Lv, pac, den, outp, f"p{b+1}",
                    )
    nc.compile()
    return nc


_PROGRAMS = {}


def _get_program(Lv, Nq):
    if (Lv, Nq) not in _PROGRAMS:
        _PROGRAMS[(Lv, Nq)] = build_program(Lv, Nq)
    return _PROGRAMS[(Lv, Nq)]


def _prep_side(x, idx, n, Lv):
    """Compact one (batch, side) to its valid rows.

    Returns (xT [128, 2*Lv] d-major halves, xA [128, NC*NAUG] chunk-row
    layout with two trailing valid-indicator columns)."""
    NC = Lv // 128
    xc = np.zeros((Lv, NAUG), np.float32)
    xc[:n, :D] = x[idx]
    xc[:n, D:] = 1.0
    t = xc[:, :D].reshape(Lv, DC, 128).transpose(1, 2, 0)  # [dc, d, row]
    xA = np.ascontiguousarray(
        xc.reshape(NC, 128, NAUG).transpose(1, 0, 2).reshape(128, NC * NAUG)
    )
    return t, xA


def run(premise_batch, premise_mask, hypothesis_batch, hypothesis_mask, trace=False):
    pb = np.asarray(premise_batch, dtype=np.float32)
    hb = np.asarray(hypothesis_batch, dtype=np.float32)
    pm = np.asarray(premise_mask)
    hm = np.asarray(hypothesis_mask)

    idx_p = [np.flatnonzero(pm[b]) for b in range(B)]
    idx_h = [np.flatnonzero(hm[b]) for b in range(B)]
    n_p = [len(i) for i in idx_p]
    n_h = [len(i) for i in idx_h]
    n_max = max(max(n_p), max(n_h), 128)
    Lv = ((n_max + 383) // 384) * 384  # multiple of 384 so NC % GRP == 0
    Nq = min(((max(max(n_p), 1) + 15) // 16) * 16, Lv)  # score query extent
    nc = _get_program(Lv, Nq)

    NC = Lv // 128
    ldA = np.empty((B, 128, 4 * Lv), BF16NP)
    ldB = np.empty((B, 128, 2 * NC * NAUG), BF16NP)
    for b in range(B):
        tp_, xAp = _prep_side(pb[b], idx_p[b], n_p[b], Lv)
        th_, xAh = _prep_side(hb[b], idx_h[b], n_h[b], Lv)
        ldA[b, :, 0 * Lv : 1 * Lv] = th_[0]
        ldA[b, :, 1 * Lv : 2 * Lv] = tp_[0]
        ldA[b, :, 2 * Lv : 3 * Lv] = th_[1]
        ldA[b, :, 3 * Lv : 4 * Lv] = tp_[1]
        ldB[b, :, 0 : NC * NAUG] = xAh
        ldB[b, :, NC * NAUG :] = xAp

    in_maps = []
    for c in range(NCORES):
        s = slice(c * BPC, (c + 1) * BPC)
        in_maps.append({"ldA": ldA[s], "ldB": ldB[s]})
    res = None
    for attempt in range(3):
        try:
            res = run_bass_kernel_spmd(nc, in_maps, list(range(NCORES)), trace=trace)
            break
        except Exception:
            # Transient device wedges (NRT_EXEC_UNIT_UNRECOVERABLE etc.)
            # usually clear on re-execution.
            if attempt == 2:
                raise
    ocp = np.concatenate([res.results[c]["out_prem"] for c in range(NCORES)], axis=0)
    och = np.concatenate([res.results[c]["out_hyp"] for c in range(NCORES)], axis=0)
    out_p = np.zeros((B, L, D), np.float32)
    out_h = np.zeros((B, L, D), np.float32)
    for b in range(B):
        cp = ocp[b].reshape(128, NC, D).transpose(1, 0, 2).reshape(Lv, D)
        ch = och[b].reshape(128, NC, D).transpose(1, 0, 2).reshape(Lv, D)
        out_p[b, idx_p[b]] = cp[: n_p[b]]
        out_h[b, idx_h[b]] = ch[: n_h[b]]
    return (out_p, out_h), res


def kernel(premise_batch, premise_mask, hypothesis_batch, hypothesis_mask):
    outs, _ = run(premise_batch, premise_mask, hypothesis_batch, hypothesis_mask)
    return outs
